# revision 6
# baseline (speedup 1.0000x reference)
"""Multi-head self-attention (B=4, S=2048, E=1024, H=16) + residual + layernorm
on 8 Trainium2 NeuronCores.

Sharding: data-parallel over batch (4) x query-split (2-way) = 8 cores, each
core computing ALL 16 heads for one batch sample and half (1024) of the query
rows; K/V projections duplicated across the pair => no collective.

vs the bf16 baseline:
- All GEMMs except the score matmuls run in fp8e4 with perf_mode=DoubleRow
  (2 contraction planes per pass => half the PE passes): Q/K/V projections,
  PV (probs @ V), and WO.  Weights are host-scaled by 64 to sit in e4m3's
  normal range; descales fold into existing DVE passes.
- The softmax numerator exp() is written straight to fp8 by ScalarE; the
  denominator still rides V's augmented ones-column (scaled exp(mask)/2).
- V-bias and WO-bias are folded into the residual host-side (softmax rows
  sum to 1, so a constant V shift adds (WV_b @ WO_w.T) to the output).
- rstd = exp(-0.5*ln(var)) so ScalarE only ever needs the ln+exp table set
  (the baseline's Sqrt forced a ~1.4us ACT table swap 16x per iteration).
- Projection work is fed into the attention block loop a few PSUM-groups
  per quad instead of running as a serial PE-only phase, so TensorE fills
  the gaps of the ScalarE(exp)-bound steady state instead of making ACT
  wait ~130us up front.
"""
import numpy as np
import ml_dtypes

B, S, E = 4, 2048, 1024
H, D = 16, 64
SQ = S // 2            # query rows per core
N_CORES = 8

_CACHE = {}


def _build_nc(unroll=1, feed_rate=2):
    import concourse.bass as bass
    import concourse.mybir as mybir
    import concourse.tile as tile
    from concourse import bacc

    F32 = mybir.dt.float32
    BF16 = mybir.dt.bfloat16
    FP8 = mybir.dt.float8e4
    AF = mybir.ActivationFunctionType
    ALU = mybir.AluOpType
    DR = mybir.MatmulPerfMode.DoubleRow

    nc = bacc.Bacc("TRN2", target_bir_lowering=False, debug=False,
                   num_devices=N_CORES)

    # ---- external inputs (per-core shards, host-prepared)
    xT = nc.declare_dram_parameter("xT", [E, S], FP8, isOutput=False)
    xqT = nc.declare_dram_parameter("xqT", [E, SQ], FP8, isOutput=False)
    x_res = nc.declare_dram_parameter("x_res", [SQ, E], F32, isOutput=False)
    wqT = nc.declare_dram_parameter("wqT", [E, E], FP8, isOutput=False)
    wkT = nc.declare_dram_parameter("wkT", [E, E], FP8, isOutput=False)
    wvT = nc.declare_dram_parameter("wvT", [E, E], FP8, isOutput=False)
    woT = nc.declare_dram_parameter("woT", [E, E], FP8, isOutput=False)
    bq64 = nc.declare_dram_parameter("bq64", [128, 8], F32, isOutput=False)
    bk64 = nc.declare_dram_parameter("bk64", [128, 8], F32, isOutput=False)
    expm_t = nc.declare_dram_parameter("expm_t", [128, 16], F32, isOutput=False)
    ln_w_row = nc.declare_dram_parameter("ln_w_row", [1, E], BF16, isOutput=False)
    ln_b_row = nc.declare_dram_parameter("ln_b_row", [1, E], BF16, isOutput=False)

    out_half = nc.declare_dram_parameter("out_half", [SQ, E], F32,
                                         isOutput=True)

    def bc_ap(param, n):
        # broadcast a [1, n] dram row across 128 partitions
        return bass.AP(tensor=param, offset=0, ap=[[0, 128], [1, n]])

    with tile.TileContext(nc) as tc:
        with tc.tile_pool(name="persist", bufs=1) as pp, \
             tc.tile_pool(name="psum", bufs=2, space="PSUM") as ps, \
             tc.tile_pool(name="small", bufs=2) as sp:

          for _rep in range(unroll):
            pfx = f"r{_rep}_"

            # ---------- small constants ----------
            bq_t = pp.tile([128, 8], F32, tag="bq")
            nc.sync.dma_start(out=bq_t[:], in_=bq64.ap())
            bk_t = pp.tile([128, 8], F32, tag="bk")
            nc.sync.dma_start(out=bk_t[:], in_=bk64.ap())
            em_t = pp.tile([128, 16], F32, tag="em")
            nc.sync.dma_start(out=em_t[:], in_=expm_t.ap())
            ones_row = pp.tile([1, 64], BF16, tag="ones_row")
            nc.vector.memset(ones_row[:], 1.0)
            ones16 = pp.tile([128, 16], BF16, tag="ones16")
            nc.vector.memset(ones16[:], 1.0)

            # persistent activations
            q_t = pp.tile([128, 8, SQ], BF16, tag="Q")       # [p, mt, s1]
            k_t = pp.tile([128, 8, S], BF16, tag="K")        # [p, mt, s2]
            v8_t = pp.tile([128, 16, 16, 65], FP8, tag="V")  # [s2p, s2t, h, d+1]
            ctx8_t = pp.tile([128, 8, SQ], FP8, tag="ctx")   # [p(m), mt, s1]

            # ---------- weight/x loads ----------
            with tc.tile_pool(name="w1", bufs=1) as w1:
                xT_t = w1.tile([128, 8, S], FP8, tag="xT")
                xqT_t = w1.tile([128, 8, SQ], FP8, tag="xqT")
                wq_t = w1.tile([128, 8, E], FP8, tag="wq")
                wk_t = w1.tile([128, 8, E], FP8, tag="wk")
                wv_t = w1.tile([128, 8, E], FP8, tag="wv")
                wo_t = w1.tile([128, 8, E], FP8, tag="wo")
                for kt in range(8):
                    nc.sync.dma_start(
                        out=wk_t[:, kt, :],
                        in_=wkT.ap().rearrange("(kt p) m -> p kt m",
                                               p=128)[:, kt, :])
                    nc.sync.dma_start(
                        out=xT_t[:, kt, :],
                        in_=xT.ap().rearrange("(kt p) s -> p kt s",
                                              p=128)[:, kt, :])
                for kt in range(8):
                    nc.sync.dma_start(
                        out=wq_t[:, kt, :],
                        in_=wqT.ap().rearrange("(kt p) m -> p kt m",
                                               p=128)[:, kt, :])
                    nc.sync.dma_start(
                        out=xqT_t[:, kt, :],
                        in_=xqT.ap().rearrange("(kt p) s -> p kt s",
                                               p=128)[:, kt, :])
                    nc.sync.dma_start(
                        out=wv_t[:, kt, :],
                        in_=wvT.ap().rearrange("(kt p) m -> p kt m",
                                               p=128)[:, kt, :])
                nc.sync.dma_start(out=wo_t[:], in_=woT.ap().rearrange(
                    "(mt p) eo -> p mt eo", p=128))

                lnw_bc = w1.tile([128, E], BF16, tag="lnw_bc")
                nc.sync.dma_start(out=lnw_bc[:], in_=bc_ap(ln_w_row, E))
                lnb_bc = w1.tile([128, E], BF16, tag="lnb_bc")
                nc.sync.dma_start(out=lnb_bc[:], in_=bc_ap(ln_b_row, E))

                # ---------- projection emitters (DoubleRow fp8) ----------
                def emit_k(mt, sb):
                    p = ps.tile([128, 512], F32, tag="mm")
                    for c in range(4):
                        nc.tensor.matmul(
                            p[:],
                            wk_t[:, 2 * c:2 * c + 2, mt * 128:(mt + 1) * 128],
                            xT_t[:, 2 * c:2 * c + 2, sb * 512:(sb + 1) * 512],
                            start=(c == 0), stop=(c == 3), perf_mode=DR)
                    # k = psum/64 + bk  ==  (psum + bk64) * (1/64)
                    nc.vector.tensor_scalar(
                        out=k_t[:, mt, sb * 512:(sb + 1) * 512],
                        in0=p[:], scalar1=bk_t[:, mt:mt + 1],
                        scalar2=0.015625, op0=ALU.add, op1=ALU.mult)

                def emit_q(mt, sb):
                    p = ps.tile([128, 512], F32, tag="mm")
                    for c in range(4):
                        nc.tensor.matmul(
                            p[:],
                            wq_t[:, 2 * c:2 * c + 2, mt * 128:(mt + 1) * 128],
                            xqT_t[:, 2 * c:2 * c + 2, sb * 512:(sb + 1) * 512],
                            start=(c == 0), stop=(c == 3), perf_mode=DR)
                    nc.vector.tensor_scalar(
                        out=q_t[:, mt, sb * 512:(sb + 1) * 512],
                        in0=p[:], scalar1=bq_t[:, mt:mt + 1],
                        scalar2=0.015625, op0=ALU.add, op1=ALU.mult)

                def emit_v(s2t, half):
                    # v8 = (x@Wv*64) * (exp(mask)/2)  => 32*v*exp(mask)
                    p = ps.tile([128, 512], F32, tag="mm")
                    for c in range(4):
                        nc.tensor.matmul(
                            p[:],
                            xT_t[:, 2 * c:2 * c + 2,
                                 s2t * 128:(s2t + 1) * 128],
                            wv_t[:, 2 * c:2 * c + 2,
                                 half * 512:(half + 1) * 512],
                            start=(c == 0), stop=(c == 3), perf_mode=DR)
                    nc.vector.tensor_scalar_mul(
                        out=v8_t[:, s2t, half * 8:(half + 1) * 8, 0:64],
                        in0=p[:].rearrange("p (h d) -> p h d", h=8),
                        scalar1=em_t[:, s2t:s2t + 1])
                    if half == 0:
                        # denominator column = exp(mask)/2 for all 16 heads
                        nc.vector.tensor_scalar_mul(
                            out=v8_t[:, s2t, :, 64],
                            in0=ones16[:], scalar1=em_t[:, s2t:s2t + 1])

                def kq_items(mt):
                    return ([lambda m=mt, s=sb: emit_k(m, s)
                             for sb in range(4)]
                            + [lambda m=mt, s=sb: emit_q(m, s)
                               for sb in range(2)])

                # lead-in: K/Q for head-pair 0 only; the rest feeds the
                # attention loop.
                for sb in range(4):
                    emit_k(0, sb)
                emit_q(0, 0)
                emit_q(0, 1)

                # deadline order (drained `feed_rate` per quad): K/Q of
                # head-pair mt must be ready by block mt; V half0 by the pv
                # stream of block 1; V half1 by block 5.
                pe_feed = []
                pe_feed += kq_items(1)
                pe_feed += [lambda t=s2t: emit_v(t, 0) for s2t in range(16)]
                pe_feed += kq_items(2)
                pe_feed += kq_items(3)
                pe_feed += [lambda t=s2t: emit_v(t, 1) for s2t in range(16)]
                for mt in range(4, 8):
                    pe_feed += kq_items(mt)

                # ---------- attention + fused WO/LN, pipelined ----------
                blocks = [(sb1, hm) for sb1 in range(2) for hm in range(8)]
                state = {}

                def emit_scores_quad(i, q):
                    sb1, hm = blocks[i]
                    st = ps.tile([128, 4, 512], F32, tag="st", bufs=1,
                                 name=f"st{pfx}{i}_{q}")
                    s1 = slice(sb1 * 512, (sb1 + 1) * 512)
                    for j in range(2):
                        for idx, hp in enumerate((0, 64)):
                            s2t = 2 * q + j
                            nc.tensor.matmul(
                                st[:, 2 * idx + j, :],
                                k_t[hp:hp + 64, hm,
                                    s2t * 128:(s2t + 1) * 128],
                                q_t[hp:hp + 64, hm, s1],
                                start=True, stop=True, tile_position=(hp, 0))
                    exp_pair = state[i]["exp"]
                    nc.scalar.activation(
                        out=exp_pair[:, 2 * q:2 * q + 2, :, :].rearrange(
                            "p t h f -> p h t f"),
                        in_=st[:].rearrange("p (h t) f -> p h t f", h=2),
                        func=AF.Exp, scale=0.125)

                def emit_pv_quad(i, q):
                    # DoubleRow: one matmul covers the s2t pair (2q, 2q+1)
                    exp_pair = state[i]["exp"]
                    pvs = state[i]["pv"]
                    for idx in range(2):
                        hl = blocks[i][1] * 2 + idx
                        nc.tensor.matmul(
                            pvs[idx][:],
                            v8_t[:, 2 * q:2 * q + 2, hl, :],
                            exp_pair[:, 2 * q:2 * q + 2, idx, :],
                            start=(q == 0), stop=(q == 7), perf_mode=DR)

                def emit_pv_norm(i):
                    sb1, hm = blocks[i]
                    s1 = slice(sb1 * 512, (sb1 + 1) * 512)
                    for idx, hp in enumerate((0, 64)):
                        pv = state[i]["pv"][idx]
                        den = sp.tile([1, 512], BF16, tag="den",
                                      name=f"den{pfx}{i}_{idx}")
                        nc.vector.tensor_copy(out=den[:], in_=pv[64:65, :])
                        bcp = ps.tile([64, 512], F32, tag="mm",
                                      name=f"bcp{pfx}{i}_{idx}")
                        nc.tensor.matmul(bcp[:], ones_row[:], den[:],
                                         start=True, stop=True)
                        rec = sp.tile([64, 512], F32, tag="rec",
                                      name=f"rec{pfx}{i}_{idx}")
                        nc.vector.reciprocal(out=rec[:], in_=bcp[:])
                        # ctx8 = (32*ctx_un) * (2/den) = 64*ctx
                        nc.vector.tensor_mul(
                            out=ctx8_t[hp:hp + 64, hm, s1],
                            in0=pv[0:64, :], in1=rec[:])

                def emit_wo_ln_tile(st_i, ep):
                    rows = slice(st_i * 128, (st_i + 1) * 128)
                    xr = ep.tile([128, E], F32, tag="xr",
                                 name=f"xr{pfx}{st_i}")
                    nc.sync.dma_start(out=xr[:], in_=x_res.ap()[rows, :])
                    v = ep.tile([128, E], F32, tag="v", name=f"v{pfx}{st_i}")
                    for eb in range(2):
                        p = ps.tile([128, 512], F32, tag="mm",
                                    name=f"wop{pfx}{st_i}_{eb}")
                        for c in range(4):
                            nc.tensor.matmul(
                                p[:],
                                ctx8_t[:, 2 * c:2 * c + 2,
                                       st_i * 128:(st_i + 1) * 128],
                                wo_t[:, 2 * c:2 * c + 2,
                                     eb * 512:(eb + 1) * 512],
                                start=(c == 0), stop=(c == 3), perf_mode=DR)
                        # v = psum/4096 + (x + bo + bv@Wo.T)
                        nc.vector.scalar_tensor_tensor(
                            out=v[:, eb * 512:(eb + 1) * 512], in0=p[:],
                            scalar=1.0 / 4096.0,
                            in1=xr[:, eb * 512:(eb + 1) * 512],
                            op0=ALU.mult, op1=ALU.add)
                    stats = ep.tile([128, 2, 6], F32, tag="stats",
                                    name=f"stats{pfx}{st_i}")
                    nc.vector.bn_stats(out=stats[:, 0, :], in_=v[:, 0:512])
                    nc.vector.bn_stats(out=stats[:, 1, :], in_=v[:, 512:1024])
                    mv = ep.tile([128, 2], F32, tag="mv",
                                 name=f"mv{pfx}{st_i}")
                    nc.vector.bn_aggr(out=mv[:], in_=stats[:])
                    # rstd = exp(-0.5*ln(var)); keeps ACT on the ln+exp table
                    lnv = ep.tile([128, 1], F32, tag="lnv",
                                  name=f"lnv{pfx}{st_i}")
                    nc.scalar.activation(out=lnv[:], in_=mv[:, 1:2],
                                         func=AF.Ln)
                    rstd = ep.tile([128, 1], F32, tag="rstd",
                                   name=f"rstd{pfx}{st_i}")
                    nc.scalar.activation(out=rstd[:], in_=lnv[:],
                                         func=AF.Exp, scale=-0.5)
                    # y = ((v - mu) * lnw) * rstd + lnb
                    t = ep.tile([128, E], F32, tag="t", name=f"t{pfx}{st_i}")
                    nc.vector.scalar_tensor_tensor(
                        out=t[:], in0=v[:], scalar=mv[:, 0:1],
                        in1=lnw_bc[:], op0=ALU.subtract, op1=ALU.mult)
                    nc.vector.scalar_tensor_tensor(
                        out=v[:], in0=t[:], scalar=rstd[:, 0:1],
                        in1=lnb_bc[:], op0=ALU.mult, op1=ALU.add)
                    nc.sync.dma_start(out=out_half.ap()[rows, :], in_=v[:])

                with tc.tile_pool(name="epi", bufs=2) as ep:
                    wo_queue = []
                    for i in range(len(blocks) + 1):
                        if i < len(blocks):
                            state[i] = {
                                "exp": pp.tile([128, 16, 2, 512], FP8,
                                               tag="exp", bufs=2,
                                               name=f"exp{pfx}{i}"),
                                "pv": [ps.tile([65, 512], F32, tag="pv",
                                               bufs=2,
                                               name=f"pv{pfx}{i}_{idx}")
                                       for idx in range(2)],
                            }
                        for q in range(8):
                            if i < len(blocks):
                                emit_scores_quad(i, q)
                            if i > 0:
                                emit_pv_quad(i - 1, q)
                            for _ in range(feed_rate):
                                if pe_feed:
                                    pe_feed.pop(0)()
                            if wo_queue and q % 2 == 1:
                                wo_queue.pop(0)()
                        if i > 0:
                            emit_pv_norm(i - 1)
                            state.pop(i - 1)
                            if blocks[i - 1][1] == 7:
                                sb1 = blocks[i - 1][0]
                                wo_queue.extend(
                                    [lambda s=sb1 * 4 + ti: emit_wo_ln_tile(
                                        s, ep) for ti in range(4)])
                    for fn in wo_queue:
                        fn()

    nc.finalize()
    return nc


def _prepare_in_maps(inputs):
    f8 = ml_dtypes.float8_e4m3fn
    f32 = np.float32
    x = np.ascontiguousarray(inputs["input_tensor"], dtype=f32)
    mask = np.ascontiguousarray(inputs["mask"], dtype=f32)
    WQ = np.asarray(inputs["WQ_w"], f32)
    WK = np.asarray(inputs["WK_w"], f32)
    WV = np.asarray(inputs["WV_w"], f32)
    WO = np.asarray(inputs["WO_w"], f32)
    # V bias and WO bias fold into the residual: probs rows sum to 1 so a
    # constant V shift contributes bv @ WO.T to every output row.
    res_bias = (np.asarray(inputs["WO_b"], f32)
                + np.asarray(inputs["WV_b"], f32) @ WO.T)
    in_maps = []
    for c in range(N_CORES):
        b, hc = divmod(c, 2)
        m = {
            "xT": np.ascontiguousarray(x[b].T).astype(f8),
            "xqT": np.ascontiguousarray(
                x[b, hc * SQ:(hc + 1) * SQ].T).astype(f8),
            "x_res": np.ascontiguousarray(
                x[b, hc * SQ:(hc + 1) * SQ] + res_bias),
            "wqT": np.ascontiguousarray(WQ.T * 64.0).astype(f8),
            "wkT": np.ascontiguousarray(WK.T * 64.0).astype(f8),
            "wvT": np.ascontiguousarray(WV.T * 64.0).astype(f8),
            "woT": np.ascontiguousarray(WO.T * 64.0).astype(f8),
            "bq64": np.ascontiguousarray(
                np.asarray(inputs["WQ_b"], f32).reshape(8, 128).T * 64.0),
            "bk64": np.ascontiguousarray(
                np.asarray(inputs["WK_b"], f32).reshape(8, 128).T * 64.0),
            "expm_t": np.ascontiguousarray(
                (np.exp(mask[b, 0, 0]) * 0.5).reshape(16, 128).T.astype(f32)),
            "ln_w_row": np.asarray(
                inputs["ln_w"], f32).reshape(1, E).astype(ml_dtypes.bfloat16),
            "ln_b_row": np.asarray(
                inputs["ln_b"], f32).reshape(1, E).astype(ml_dtypes.bfloat16),
        }
        in_maps.append({k: np.ascontiguousarray(v) for k, v in m.items()})
    return in_maps


def _run(inputs, trace=False):
    from concourse.bass_utils import run_bass_kernel_spmd

    if "nc" not in _CACHE:
        _CACHE["nc"] = _build_nc()
    in_maps = _prepare_in_maps(inputs)
    res = run_bass_kernel_spmd(_CACHE["nc"], in_maps, list(range(N_CORES)),
                               trace=trace)
    out = np.empty((B, S, E), np.float32)
    for c in range(N_CORES):
        b, hc = divmod(c, 2)
        out[b, hc * SQ:(hc + 1) * SQ] = res.results[c]["out_half"]
    return out, res


def kernel(**inputs):
    out, _ = _run(inputs, trace=False)
    return out
